# revision 21
# baseline (speedup 1.0000x reference)
"""Bass/Tile TRN2 kernel for BlenderbotSelfAttention decode step.

B=16, QLEN=1, DMODEL=2048, NHEAD=32, HEAD_DIM=64, SKV=4096.
Sharding: tensor-parallel over heads -- 4 heads per core on 8 cores.
Heads are processed in pairs with their head_dim=64 vectors stacked to
K=128, and the big KV-cache matmuls stream the cache through the PE
moving-operand port as float32r with N>=256 (1 cycle/row), never through
LDWEIGHTS.  Probabilities are transposed from row layout back to column
layout with a small DRAM round-trip per (batch, pair).
Row-parallel out_proj partials are summed on host (all-reduce equivalent).
"""

import numpy as np

B = 16
DMODEL = 2048
NHEAD = 32
HD = 64
SKV = 4096
NCORES = 8
HLOC = NHEAD // NCORES      # 4 heads per core
NPAIR = HLOC // 2           # 2 pairs per core
NBP = B * NPAIR             # 32 (batch, pair) units per core
NCH = SKV // 128            # 32 seq chunks of 128

_PROG = None
# stream-matmul operand mode: float32r (TF32-like, 11-bit mantissa, 1 cyc/row
# at N>=256) vs plain float32 (4 cyc/row). f32r needs host-side pre-rounding.
import os as _os
STREAM_F32R = _os.environ.get("KSTREAM", "f32r") == "f32r"


def _round_f32r(a):
    """Round-to-nearest-even to 11-bit mantissa (fp32r), fp32-bit-compatible."""
    u = np.ascontiguousarray(a, np.float32).view(np.uint32)
    r = (u + np.uint32(0x7FF) + ((u >> np.uint32(12)) & np.uint32(1))) & np.uint32(0xFFFFF000)
    return r.view(np.float32)


def _build():
    """Build + compile the per-core Bass program (identical on all cores)."""
    from contextlib import ExitStack

    import concourse.bacc as bacc
    import concourse.mybir as mybir
    import concourse.tile as tile

    f32 = mybir.dt.float32
    f32r = mybir.dt.float32r
    STR = f32r if STREAM_F32R else f32
    EXP = mybir.ActivationFunctionType.Exp
    nc = bacc.Bacc("TRN2", target_bir_lowering=False, debug=False,
                   enable_asserts=False, num_devices=NCORES)

    xT = nc.dram_tensor("xT", [DMODEL, B], f32, kind="ExternalInput").ap()
    wqk = nc.dram_tensor("wqk", [DMODEL, 512], f32, kind="ExternalInput").ap()
    bqk = nc.dram_tensor("bqk", [128, 4], f32, kind="ExternalInput").ap()
    wv = nc.dram_tensor("wv", [DMODEL, 256], f32, kind="ExternalInput").ap()
    bv = nc.dram_tensor("bv", [1, 256], f32, kind="ExternalInput").ap()
    wo = nc.dram_tensor("wo", [256, DMODEL], f32, kind="ExternalInput").ap()
    kt2 = nc.dram_tensor("kt2", [NBP, 128, SKV], STR, kind="ExternalInput").ap()
    vz = nc.dram_tensor("vz", [B, SKV, 256], STR, kind="ExternalInput").ap()
    m1 = nc.dram_tensor("m1", [B, 128, 128], STR, kind="ExternalInput").ap()
    ones128 = nc.dram_tensor("ones128", [128, 1], f32, kind="ExternalInput").ap()
    sel4 = nc.dram_tensor("sel4", [128, 4], f32, kind="ExternalInput").ap()
    ones16 = nc.dram_tensor("ones16", [1, B], f32, kind="ExternalInput").ap()
    u01 = nc.dram_tensor("u01", [1, 256], f32, kind="ExternalInput").ap()
    eye4 = nc.dram_tensor("eye4", [4, 4], f32, kind="ExternalInput").ap()

    y = nc.dram_tensor("y", [B, DMODEL], f32, kind="ExternalOutput").ap()
    knew = nc.dram_tensor("knew", [256, B], f32, kind="ExternalOutput").ap()
    vnew = nc.dram_tensor("vnew", [B, 256], f32, kind="ExternalOutput").ap()

    NK = DMODEL // 128  # 16 contraction chunks for the projections

    with tile.TileContext(nc) as tc, ExitStack() as ctx:
        cpool = ctx.enter_context(tc.tile_pool(name="cpool", bufs=1))

        # ---- small resident constants ----
        wo_sb = cpool.tile([128, 2 * DMODEL], f32, tag="wo", name="wo")
        nc.sync.dma_start(
            wo_sb[:].rearrange("p (kc f) -> p kc f", kc=2),
            wo.rearrange("(kc p) f -> p kc f", p=128))
        bqk_sb = cpool.tile([128, 4], f32, tag="bqk", name="bqk")
        nc.sync.dma_start(bqk_sb[:], bqk)
        bv_sb = cpool.tile([1, 256], f32, tag="bv", name="bv")
        nc.sync.dma_start(bv_sb[:], bv)
        m1_sb = cpool.tile([128, B * 128], STR, tag="m1", name="m1")
        nc.sync.dma_start(
            m1_sb[:].rearrange("p (b e) -> p b e", b=B),
            m1.rearrange("b p e -> p b e"))
        ones128_sb = cpool.tile([128, 1], f32, tag="ones128", name="ones128")
        nc.sync.dma_start(ones128_sb[:], ones128)
        sel4_sb = cpool.tile([128, 4], f32, tag="sel4", name="sel4")
        nc.sync.dma_start(sel4_sb[:], sel4)
        ones16_sb = cpool.tile([1, B], f32, tag="ones16", name="ones16")
        nc.sync.dma_start(ones16_sb[:], ones16)
        u01_sb = cpool.tile([1, 256], f32, tag="u01", name="u01")
        nc.sync.dma_start(u01_sb[:], u01)
        eye4_sb = cpool.tile([4, 4], f32, tag="eye4", name="eye4")
        nc.sync.dma_start(eye4_sb[:], eye4)

        # persistent intermediates
        qk_sb = [cpool.tile([128, B], f32, tag=f"qk{ft}", name=f"qk{ft}")
                 for ft in range(4)]
        v_sb = cpool.tile([B, 256], f32, tag="vsb", name="vsb")
        q4 = [cpool.tile([128, 4 * B], STR, tag=f"q4_{p}", name=f"q4_{p}")
              for p in range(NPAIR)]
        at_sb = [cpool.tile([128, B], f32, tag=f"at{p}", name=f"at{p}")
                 for p in range(NPAIR)]
        dall_sb = cpool.tile([1, 2 * NBP], f32, tag="dall", name="dall")
        r_sb = cpool.tile([1, 2 * NBP], f32, tag="rall", name="rall")
        atn_sb = [cpool.tile([128, B], f32, tag=f"atn{p}", name=f"atn{p}")
                  for p in range(NPAIR)]
        y_sb = cpool.tile([B, DMODEL], f32, tag="ysb", name="ysb")

        # ---- phase 1: projections (plain fp32 for exact k/v/q) ----
        with tc.tile_pool(name="p1", bufs=1) as p1, \
             tc.tile_pool(name="pps", bufs=2, space="PSUM") as pps, \
             tc.tile_pool(name="pvs", bufs=1, space="PSUM") as pvs:
            xt_sb = p1.tile([128, NK * B], f32, tag="xt", name="xt")
            nc.sync.dma_start(
                xt_sb[:].rearrange("p (kc f) -> p kc f", kc=NK),
                xT.rearrange("(kc p) f -> p kc f", p=128))
            wqk_sb = p1.tile([128, NK * 512], f32, tag="wqk", name="wqk")
            nc.sync.dma_start(
                wqk_sb[:].rearrange("p (kc f) -> p kc f", kc=NK),
                wqk.rearrange("(kc p) f -> p kc f", p=128))
            wv_sb = p1.tile([128, NK * 256], f32, tag="wv", name="wv")
            nc.sync.dma_start(
                wv_sb[:].rearrange("p (kc f) -> p kc f", kc=NK),
                wv.rearrange("(kc p) f -> p kc f", p=128))

            # q/k column orientation: psum[feat128, b]; ft 0,1 = q pair0/1
            # (scaled 1/8 host-side), ft 2,3 = k pair0/1
            for ft in range(4):
                ps = pps.tile([128, B], f32, tag="qkps", name="qkps")
                for kc in range(NK):
                    nc.tensor.matmul(
                        ps[:],
                        wqk_sb[:, kc * 512 + ft * 128: kc * 512 + (ft + 1) * 128],
                        xt_sb[:, kc * B:(kc + 1) * B],
                        start=(kc == 0), stop=(kc == NK - 1))
                nc.vector.tensor_scalar_add(qk_sb[ft][:], ps[:],
                                            bqk_sb[:, ft:ft + 1])
            # v row orientation: psum[b, feat256]; bias via ones-row matmul
            psv = pvs.tile([B, 256], f32, tag="vps", name="vps")
            for kc in range(NK):
                nc.tensor.matmul(psv[:], xt_sb[:, kc * B:(kc + 1) * B],
                                 wv_sb[:, kc * 256:(kc + 1) * 256],
                                 start=(kc == 0), stop=False)
            nc.tensor.matmul(psv[:], ones16_sb[:], bv_sb[:],
                             start=False, stop=True)
            nc.vector.tensor_copy(v_sb[:], psv[:])

        nc.sync.dma_start(knew[0:128, :], qk_sb[2][:])
        nc.sync.dma_start(knew[128:256, :], qk_sb[3][:])
        nc.sync.dma_start(vnew[:], v_sb[:])

        # 4-row block-diagonal q: q4[p] col 4b+(2p+h) carries q of (pair p,
        # head h) on its 64-row d block; all other columns stay zero so the
        # two pairs' score matmuls can accumulate into one [4, 512] psum.
        for p in range(NPAIR):
            nc.vector.memset(q4[p][:].bitcast(f32), 0.0)
            srcq = qk_sb[p]
            d0 = q4[p][:].rearrange("p (b four) -> p b four", four=4)
            nc.vector.tensor_copy(d0[0:64, :, 2 * p:2 * p + 1],
                                  srcq[0:64, :].rearrange("p b -> p b ()"))
            nc.vector.tensor_copy(d0[64:128, :, 2 * p + 1:2 * p + 2],
                                  srcq[64:128, :].rearrange("p b -> p b ()"))

        # ---- phase 2: attention, one batch (both head pairs) at a time ----
        with tc.tile_pool(name="kp", bufs=3) as kp, \
             tc.tile_pool(name="vp", bufs=2) as vp, \
             tc.tile_pool(name="ep", bufs=2) as ep, \
             tc.tile_pool(name="e2", bufs=2) as e2, \
             tc.tile_pool(name="vnp", bufs=2) as vnp, \
             tc.tile_pool(name="scp", bufs=3, space="PSUM") as scp, \
             tc.tile_pool(name="smp", bufs=1, space="PSUM") as smp, \
             tc.tile_pool(name="pvp", bufs=2, space="PSUM") as pvp, \
             tc.tile_pool(name="trp", bufs=2, space="PSUM") as trp:
            for b in range(B):
                v_t = vp.tile([128, NCH * 256], STR, tag="vt", name="vt")
                nc.sync.dma_start(
                    v_t[:].rearrange("p (c w) -> p c w", c=NCH),
                    vz[b].rearrange("(c p) w -> p c w", p=128))
                vn_row = vnp.tile([1, 256], f32, tag="vnr", name="vnr")
                nc.sync.dma_start(vn_row[:], v_sb[b:b + 1, :])

                # e4 rows: (pair, h); scores land here as exp(score)
                e4 = ep.tile([4, SKV], f32, tag="e4", name="e4")
                pT = e2.tile([128, 128], STR, tag="pT", name="pT")
                exv = e2.tile([1, 4], f32, tag="exv", name="exv")
                kt_t = [None, None]
                for pair in range(NPAIR):
                    kt_t[pair] = kp.tile([128, SKV], STR, tag="kt", name="kt")
                    nc.sync.dma_start(kt_t[pair][:], kt2[2 * b + pair])
                lq = [q4[p][:, 4 * b:4 * b + 4] for p in range(NPAIR)]
                for j in range(8):
                    sc = scp.tile([4, 512], f32, tag="sc", name="sc")
                    for pair in range(NPAIR):
                        nc.tensor.matmul(sc[:], lq[pair],
                                         kt_t[pair][:, j * 512:(j + 1) * 512],
                                         start=(pair == 0), stop=(pair == 1))
                    nc.scalar.activation(e4[:, j * 512:(j + 1) * 512],
                                         sc[:], EXP)
                # new-token scores for both pairs -> one [1, 4] row
                ex_ps = smp.tile([1, 4], f32, tag="sm", name="exps")
                for pair in range(NPAIR):
                    lqf = lq[pair].bitcast(f32) if STREAM_F32R else lq[pair]
                    nc.tensor.matmul(ex_ps[:], qk_sb[2 + pair][:, b:b + 1],
                                     lqf, start=(pair == 0), stop=(pair == 1))
                nc.scalar.activation(exv[:], ex_ps[:], EXP)

                # transpose probs to column layout pT[p, (c, pair, h)];
                # two chunks share one psum tile -> one copy per pair of chunks
                for c2 in range(NCH // 2):
                    tr = trp.tile([128, 8], f32, tag="tr", name="tr")
                    for k in range(2):
                        c = 2 * c2 + k
                        nc.tensor.transpose(tr[:, 4 * k:4 * k + 4],
                                            e4[:, c * 128:(c + 1) * 128],
                                            eye4_sb[:])
                    nc.vector.tensor_copy(pT[:, 8 * c2:8 * c2 + 8], tr[:])

                # mask (kills stale last_pos column and s > last_pos)
                nc.vector.tensor_mul(pT[:], pT[:],
                                     m1_sb[:, b * 128:(b + 1) * 128])

                # denominators -> dall[0, 4b + (2*pair + h)]
                cs_t = smp.tile([128, 1], f32, tag="sm", name="cst")
                nc.tensor.matmul(cs_t[:], pT[:].bitcast(f32),
                                 ones128_sb[:], start=True, stop=True)
                cs_sb = e2.tile([128, 1], f32, tag="cssb", name="cssb")
                nc.vector.tensor_copy(cs_sb[:], cs_t[:])
                dn_t = smp.tile([1, 4], f32, tag="sm", name="dnt")
                nc.tensor.matmul(dn_t[:], cs_sb[:], sel4_sb[:],
                                 start=True, stop=True)
                nc.vector.tensor_add(dall_sb[0:1, 4 * b:4 * b + 4],
                                     dn_t[:], exv[:])

                # PV: both pairs at once, N=256 streaming
                pv_t = pvp.tile([4, 256], f32, tag="pvt", name="pvt")
                for c in range(NCH):
                    nc.tensor.matmul(pv_t[:],
                                     pT[:, 4 * c:4 * c + 4],
                                     v_t[:, c * 256:(c + 1) * 256],
                                     start=(c == 0), stop=False)
                nc.tensor.matmul(pv_t[:], exv[:], vn_row[:],
                                 start=False, stop=True)

                pv_sb = e2.tile([4, 256], f32, tag="pvsb", name="pvsb")
                nc.vector.tensor_copy(pv_sb[:], pv_t[:])
                for pair in range(NPAIR):
                    tr = trp.tile([128, 4], f32, tag="tr", name="tr")
                    nc.tensor.transpose(tr[:],
                                        pv_sb[:, pair * 128:(pair + 1) * 128],
                                        eye4_sb[:])
                    nc.vector.tensor_copy(at_sb[pair][0:64, b:b + 1],
                                          tr[0:64, 2 * pair:2 * pair + 1])
                    nc.vector.tensor_copy(at_sb[pair][64:128, b:b + 1],
                                          tr[64:128, 2 * pair + 1:2 * pair + 2])

        # ---- phase 3: normalize + out-proj partial ----
        nc.vector.reciprocal(r_sb[:], dall_sb[:])
        with tc.tile_pool(name="rp", bufs=1, space="PSUM") as rp, \
             tc.tile_pool(name="yp", bufs=1, space="PSUM") as yp:
            for p in range(NPAIR):
                r_ps = rp.tile([128, B], f32, tag=f"rps{p}", name=f"rps{p}")
                for h in range(2):
                    j = 2 * p + h
                    nc.tensor.matmul(r_ps[:], u01_sb[0:1, h * 128:(h + 1) * 128],
                                     r_sb[0:1, j:j + 61:4],
                                     start=(h == 0), stop=(h == 1))
                nc.vector.tensor_mul(atn_sb[p][:], at_sb[p][:], r_ps[:])
            for n in range(4):
                y_ps = yp.tile([B, 512], f32, tag=f"yps{n}", name=f"yps{n}")
                for p in range(NPAIR):
                    nc.tensor.matmul(
                        y_ps[:], atn_sb[p][:],
                        wo_sb[:, p * DMODEL + n * 512: p * DMODEL + (n + 1) * 512],
                        start=(p == 0), stop=(p == NPAIR - 1))
                nc.vector.tensor_copy(y_sb[:, n * 512:(n + 1) * 512], y_ps[:])
        nc.sync.dma_start(y[:], y_sb[:])

    nc.compile()
    return nc


def _get_prog():
    global _PROG
    if _PROG is None:
        _PROG = _build()
    return _PROG


def _host_prep(x, last_pos, mask, Wq, bq, Wk, bk, Wv, bv, Wo, cache_k, cache_v):
    """Build the 8 per-core input maps."""
    f32 = np.float32
    x = np.asarray(x, f32).reshape(B, DMODEL)
    lp = np.asarray(last_pos).astype(np.int64)
    mask2 = np.asarray(mask).reshape(B, SKV).astype(bool)
    Wq = np.asarray(Wq, f32); Wk = np.asarray(Wk, f32)
    Wv = np.asarray(Wv, f32); Wo = np.asarray(Wo, f32)
    bq = np.asarray(bq, f32); bk = np.asarray(bk, f32); bv = np.asarray(bv, f32)

    xT = np.ascontiguousarray(x.T)
    _rnd = _round_f32r if STREAM_F32R else (lambda a: a)

    # shared mask tile [B, 128, 128]: dup over (pair, h); excludes stale
    # last_pos column (new-token prob handled separately, always valid)
    s_idx = np.arange(SKV)
    keep = (mask2 & (s_idx[None, :] != lp[:, None])).astype(f32)
    km = keep.reshape(B, NCH, 128).transpose(0, 2, 1)          # [B,128,32]
    m1 = np.ascontiguousarray(np.repeat(km, 4, axis=2))        # [B,128,(c,pp,h)]
    assert bool(np.all(mask2[np.arange(B), lp])), \
        "new-token position must be attendable"

    sel = np.zeros((128, 4), f32)
    m = np.arange(128)
    sel[m, m % 4] = 1.0
    u01 = np.zeros((1, 256), f32)
    u01[0, 0:64] = 1.0
    u01[0, 192:256] = 1.0
    consts = {
        "ones128": np.ones((128, 1), f32),
        "sel4": sel,
        "ones16": np.ones((1, B), f32),
        "u01": u01,
        "eye4": np.eye(4, dtype=f32),
    }

    in_maps = []
    for c in range(NCORES):
        fs = slice(c * 256, (c + 1) * 256)
        qb = bq[fs] * 0.125
        kb = bk[fs]
        ck = cache_k[:, 4 * c:4 * c + 4]
        cv = cache_v[:, 4 * c:4 * c + 4]
        in_maps.append({
            "xT": xT,
            "wqk": np.ascontiguousarray(
                np.concatenate([Wq[:, fs] * 0.125, Wk[:, fs]], axis=1)),
            "bqk": np.ascontiguousarray(
                np.stack([qb[:128], qb[128:], kb[:128], kb[128:]], axis=1)),
            "wv": np.ascontiguousarray(Wv[:, fs]),
            "bv": bv[fs].reshape(1, 256).copy(),
            "wo": np.ascontiguousarray(Wo[fs, :]),
            "kt2": _rnd(np.ascontiguousarray(
                ck.reshape(B, 2, 2, SKV, HD).transpose(0, 1, 2, 4, 3)
                .reshape(NBP, 128, SKV))),
            "vz": _rnd(np.ascontiguousarray(
                cv.reshape(B, 2, 2, SKV, HD).transpose(0, 3, 1, 2, 4)
                .reshape(B, SKV, 256))),
            "m1": m1,
            **consts,
        })
    return in_maps, lp


def _run(in_maps, trace=False):
    from concourse.bass_utils import run_bass_kernel_spmd
    nc = _get_prog()
    return run_bass_kernel_spmd(nc, in_maps, core_ids=list(range(NCORES)),
                                trace=trace)


def kernel(x, last_pos, mask, Wq, bq, Wk, bk, Wv, bv, Wo, bo,
           cache_k, cache_v, _trace=False, _result_holder=None):
    f32 = np.float32
    cache_k = np.asarray(cache_k, f32)
    cache_v = np.asarray(cache_v, f32)
    in_maps, lp = _host_prep(x, last_pos, mask, Wq, bq, Wk, bk, Wv, bv, Wo,
                             cache_k, cache_v)

    res = _run(in_maps, trace=_trace)
    if _result_holder is not None:
        _result_holder.append(res)

    bo = np.asarray(bo, f32)
    out = np.zeros((B, DMODEL), f32)
    k_new = np.zeros((B, NHEAD, HD), f32)
    v_new = np.zeros((B, NHEAD, HD), f32)
    for c in range(NCORES):
        r = res.results[c]
        out += r["y"]
        k_new[:, 4 * c:4 * c + 4] = (
            r["knew"].reshape(2, 2, HD, B).transpose(3, 0, 1, 2)
            .reshape(B, HLOC, HD))
        v_new[:, 4 * c:4 * c + 4] = r["vnew"].reshape(B, HLOC, HD)
    out = (out + bo).reshape(B, 1, DMODEL).astype(f32)

    up_k = cache_k.copy()
    up_v = cache_v.copy()
    bidx = np.arange(B)
    up_k[bidx, :, lp, :] = k_new
    up_v[bidx, :, lp, :] = v_new
    return out, up_k, up_v


# revision 22
# speedup vs baseline: 1.0761x; 1.0761x over previous
"""Bass/Tile TRN2 kernel for BlenderbotSelfAttention decode step.

B=16, QLEN=1, DMODEL=2048, NHEAD=32, HEAD_DIM=64, SKV=4096.
Sharding: tensor-parallel over heads -- 4 heads per core on 8 cores.
Heads are processed in pairs with their head_dim=64 vectors stacked to
K=128, and the big KV-cache matmuls stream the cache through the PE
moving-operand port as float32r with N>=256 (1 cycle/row), never through
LDWEIGHTS.  Probabilities are transposed from row layout back to column
layout with a small DRAM round-trip per (batch, pair).
Row-parallel out_proj partials are summed on host (all-reduce equivalent).
"""

import numpy as np

B = 16
DMODEL = 2048
NHEAD = 32
HD = 64
SKV = 4096
NCORES = 8
HLOC = NHEAD // NCORES      # 4 heads per core
NPAIR = HLOC // 2           # 2 pairs per core
NBP = B * NPAIR             # 32 (batch, pair) units per core
NCH = SKV // 128            # 32 seq chunks of 128

_PROG = None
# stream-matmul operand mode: float32r (TF32-like, 11-bit mantissa, 1 cyc/row
# at N>=256) vs plain float32 (4 cyc/row). f32r needs host-side pre-rounding.
import os as _os
STREAM_F32R = _os.environ.get("KSTREAM", "f32r") == "f32r"


def _round_f32r(a):
    """Round-to-nearest-even to 11-bit mantissa (fp32r), fp32-bit-compatible."""
    u = np.ascontiguousarray(a, np.float32).view(np.uint32)
    r = (u + np.uint32(0x7FF) + ((u >> np.uint32(12)) & np.uint32(1))) & np.uint32(0xFFFFF000)
    return r.view(np.float32)


def _build():
    """Build + compile the per-core Bass program (identical on all cores)."""
    from contextlib import ExitStack

    import concourse.bacc as bacc
    import concourse.mybir as mybir
    import concourse.tile as tile

    f32 = mybir.dt.float32
    f32r = mybir.dt.float32r
    STR = f32r if STREAM_F32R else f32
    EXP = mybir.ActivationFunctionType.Exp
    nc = bacc.Bacc("TRN2", target_bir_lowering=False, debug=False,
                   enable_asserts=False, num_devices=NCORES)

    xT = nc.dram_tensor("xT", [DMODEL, B], f32, kind="ExternalInput").ap()
    wqk = nc.dram_tensor("wqk", [DMODEL, 512], f32, kind="ExternalInput").ap()
    bqk = nc.dram_tensor("bqk", [128, 4], f32, kind="ExternalInput").ap()
    wv = nc.dram_tensor("wv", [DMODEL, 256], f32, kind="ExternalInput").ap()
    bv = nc.dram_tensor("bv", [1, 256], f32, kind="ExternalInput").ap()
    wo = nc.dram_tensor("wo", [256, DMODEL], f32, kind="ExternalInput").ap()
    kt2 = nc.dram_tensor("kt2", [NBP, 128, SKV], STR, kind="ExternalInput").ap()
    vz = nc.dram_tensor("vz", [B, SKV, 256], STR, kind="ExternalInput").ap()
    m1 = nc.dram_tensor("m1", [B, 128, 128], STR, kind="ExternalInput").ap()
    ones128 = nc.dram_tensor("ones128", [128, 1], f32, kind="ExternalInput").ap()
    sel4 = nc.dram_tensor("sel4", [128, 4], f32, kind="ExternalInput").ap()
    ones16 = nc.dram_tensor("ones16", [1, B], f32, kind="ExternalInput").ap()
    u01 = nc.dram_tensor("u01", [1, 256], f32, kind="ExternalInput").ap()
    eye4 = nc.dram_tensor("eye4", [4, 4], f32, kind="ExternalInput").ap()

    y = nc.dram_tensor("y", [B, DMODEL], f32, kind="ExternalOutput").ap()
    knew = nc.dram_tensor("knew", [256, B], f32, kind="ExternalOutput").ap()
    vnew = nc.dram_tensor("vnew", [B, 256], f32, kind="ExternalOutput").ap()

    NK = DMODEL // 128  # 16 contraction chunks for the projections

    with tile.TileContext(nc) as tc, ExitStack() as ctx:
        cpool = ctx.enter_context(tc.tile_pool(name="cpool", bufs=1))

        # ---- small resident constants ----
        wo_sb = cpool.tile([128, 2 * DMODEL], f32, tag="wo", name="wo")
        nc.sync.dma_start(
            wo_sb[:].rearrange("p (kc f) -> p kc f", kc=2),
            wo.rearrange("(kc p) f -> p kc f", p=128))
        bqk_sb = cpool.tile([128, 4], f32, tag="bqk", name="bqk")
        nc.sync.dma_start(bqk_sb[:], bqk)
        bv_sb = cpool.tile([1, 256], f32, tag="bv", name="bv")
        nc.sync.dma_start(bv_sb[:], bv)
        m1_sb = cpool.tile([128, B * 128], STR, tag="m1", name="m1")
        nc.sync.dma_start(
            m1_sb[:].rearrange("p (b e) -> p b e", b=B),
            m1.rearrange("b p e -> p b e"))
        ones128_sb = cpool.tile([128, 1], f32, tag="ones128", name="ones128")
        nc.sync.dma_start(ones128_sb[:], ones128)
        sel4_sb = cpool.tile([128, 4], f32, tag="sel4", name="sel4")
        nc.sync.dma_start(sel4_sb[:], sel4)
        ones16_sb = cpool.tile([1, B], f32, tag="ones16", name="ones16")
        nc.sync.dma_start(ones16_sb[:], ones16)
        u01_sb = cpool.tile([1, 256], f32, tag="u01", name="u01")
        nc.sync.dma_start(u01_sb[:], u01)
        eye4_sb = cpool.tile([4, 4], f32, tag="eye4", name="eye4")
        nc.sync.dma_start(eye4_sb[:], eye4)

        # persistent intermediates
        qk_sb = [cpool.tile([128, B], f32, tag=f"qk{ft}", name=f"qk{ft}")
                 for ft in range(4)]
        v_sb = cpool.tile([B, 256], f32, tag="vsb", name="vsb")
        q4 = [cpool.tile([128, 4 * B], STR, tag=f"q4_{p}", name=f"q4_{p}")
              for p in range(NPAIR)]
        at_sb = [cpool.tile([128, B], f32, tag=f"at{p}", name=f"at{p}")
                 for p in range(NPAIR)]
        dall_sb = cpool.tile([1, 2 * NBP], f32, tag="dall", name="dall")
        r_sb = cpool.tile([1, 2 * NBP], f32, tag="rall", name="rall")
        atn_sb = [cpool.tile([128, B], f32, tag=f"atn{p}", name=f"atn{p}")
                  for p in range(NPAIR)]
        y_sb = cpool.tile([B, DMODEL], f32, tag="ysb", name="ysb")

        # ---- phase 1: projections (plain fp32 for exact k/v/q) ----
        with tc.tile_pool(name="p1", bufs=1) as p1, \
             tc.tile_pool(name="pps", bufs=2, space="PSUM") as pps, \
             tc.tile_pool(name="pvs", bufs=1, space="PSUM") as pvs:
            xt_sb = p1.tile([128, NK * B], f32, tag="xt", name="xt")
            nc.sync.dma_start(
                xt_sb[:].rearrange("p (kc f) -> p kc f", kc=NK),
                xT.rearrange("(kc p) f -> p kc f", p=128))
            wqk_sb = p1.tile([128, NK * 512], f32, tag="wqk", name="wqk")
            nc.sync.dma_start(
                wqk_sb[:].rearrange("p (kc f) -> p kc f", kc=NK),
                wqk.rearrange("(kc p) f -> p kc f", p=128))
            wv_sb = p1.tile([128, NK * 256], f32, tag="wv", name="wv")
            nc.sync.dma_start(
                wv_sb[:].rearrange("p (kc f) -> p kc f", kc=NK),
                wv.rearrange("(kc p) f -> p kc f", p=128))

            # q/k column orientation: psum[feat128, b]; ft 0,1 = q pair0/1
            # (scaled 1/8 host-side), ft 2,3 = k pair0/1
            for ft in range(4):
                ps = pps.tile([128, B], f32, tag="qkps", name="qkps")
                for kc in range(NK):
                    nc.tensor.matmul(
                        ps[:],
                        wqk_sb[:, kc * 512 + ft * 128: kc * 512 + (ft + 1) * 128],
                        xt_sb[:, kc * B:(kc + 1) * B],
                        start=(kc == 0), stop=(kc == NK - 1))
                nc.vector.tensor_scalar_add(qk_sb[ft][:], ps[:],
                                            bqk_sb[:, ft:ft + 1])
            # v row orientation: psum[b, feat256]; bias via ones-row matmul
            psv = pvs.tile([B, 256], f32, tag="vps", name="vps")
            for kc in range(NK):
                nc.tensor.matmul(psv[:], xt_sb[:, kc * B:(kc + 1) * B],
                                 wv_sb[:, kc * 256:(kc + 1) * 256],
                                 start=(kc == 0), stop=False)
            nc.tensor.matmul(psv[:], ones16_sb[:], bv_sb[:],
                             start=False, stop=True)
            nc.vector.tensor_copy(v_sb[:], psv[:])

        nc.sync.dma_start(knew[0:128, :], qk_sb[2][:])
        nc.sync.dma_start(knew[128:256, :], qk_sb[3][:])
        nc.sync.dma_start(vnew[:], v_sb[:])

        # 4-row block-diagonal q: q4[p] col 4b+(2p+h) carries q of (pair p,
        # head h) on its 64-row d block; all other columns stay zero so the
        # two pairs' score matmuls can accumulate into one [4, 512] psum.
        for p in range(NPAIR):
            nc.vector.memset(q4[p][:].bitcast(f32), 0.0)
            srcq = qk_sb[p]
            d0 = q4[p][:].rearrange("p (b four) -> p b four", four=4)
            nc.vector.tensor_copy(d0[0:64, :, 2 * p:2 * p + 1],
                                  srcq[0:64, :].rearrange("p b -> p b ()"))
            nc.vector.tensor_copy(d0[64:128, :, 2 * p + 1:2 * p + 2],
                                  srcq[64:128, :].rearrange("p b -> p b ()"))

        # ---- phase 2: attention, one batch (both head pairs) at a time ----
        with tc.tile_pool(name="kp", bufs=3) as kp, \
             tc.tile_pool(name="vp", bufs=2) as vp, \
             tc.tile_pool(name="ep", bufs=2) as ep, \
             tc.tile_pool(name="e2", bufs=2) as e2, \
             tc.tile_pool(name="vnp", bufs=2) as vnp, \
             tc.tile_pool(name="scp", bufs=2, space="PSUM") as scp, \
             tc.tile_pool(name="smp", bufs=2, space="PSUM") as smp, \
             tc.tile_pool(name="pvp", bufs=2, space="PSUM") as pvp, \
             tc.tile_pool(name="trp", bufs=2, space="PSUM") as trp:
            for b in range(B):
                v_t = vp.tile([128, NCH * 256], STR, tag="vt", name="vt")
                nc.sync.dma_start(
                    v_t[:].rearrange("p (c w) -> p c w", c=NCH),
                    vz[b].rearrange("(c p) w -> p c w", p=128))
                vn_row = vnp.tile([1, 256], f32, tag="vnr", name="vnr")
                nc.sync.dma_start(vn_row[:], v_sb[b:b + 1, :])

                # e4 rows: (pair, h); scores land here as exp(score)
                e4 = ep.tile([4, SKV], f32, tag="e4", name="e4")
                pT = e2.tile([128, 128], STR, tag="pT", name="pT")
                exv = e2.tile([1, 4], f32, tag="exv", name="exv")
                kt_t = [None, None]
                for pair in range(NPAIR):
                    kt_t[pair] = kp.tile([128, SKV], STR, tag="kt", name="kt")
                    nc.sync.dma_start(kt_t[pair][:], kt2[2 * b + pair])
                lq = [q4[p][:, 4 * b:4 * b + 4] for p in range(NPAIR)]
                for j in range(8):
                    sc = scp.tile([4, 512], f32, tag="sc", name="sc")
                    for pair in range(NPAIR):
                        nc.tensor.matmul(sc[:], lq[pair],
                                         kt_t[pair][:, j * 512:(j + 1) * 512],
                                         start=(pair == 0), stop=(pair == 1))
                    nc.scalar.activation(e4[:, j * 512:(j + 1) * 512],
                                         sc[:], EXP)
                # new-token scores for both pairs -> one [1, 4] row
                ex_ps = smp.tile([1, 4], f32, tag="sm", name="exps")
                for pair in range(NPAIR):
                    lqf = lq[pair].bitcast(f32) if STREAM_F32R else lq[pair]
                    nc.tensor.matmul(ex_ps[:], qk_sb[2 + pair][:, b:b + 1],
                                     lqf, start=(pair == 0), stop=(pair == 1))
                nc.scalar.activation(exv[:], ex_ps[:], EXP)

                # transpose probs to column layout pT[p, (c, pair, h)]
                for c in range(NCH):
                    tr = trp.tile([128, 4], f32, tag="tr", name="tr")
                    nc.tensor.transpose(tr[:], e4[:, c * 128:(c + 1) * 128],
                                        eye4_sb[:])
                    nc.vector.tensor_copy(pT[:, 4 * c:4 * c + 4], tr[:])

                # mask (kills stale last_pos column and s > last_pos)
                nc.vector.tensor_mul(pT[:], pT[:],
                                     m1_sb[:, b * 128:(b + 1) * 128])

                # denominators -> dall[0, 4b + (2*pair + h)]
                cs_t = smp.tile([128, 1], f32, tag="sm", name="cst")
                nc.tensor.matmul(cs_t[:], pT[:].bitcast(f32),
                                 ones128_sb[:], start=True, stop=True)
                cs_sb = e2.tile([128, 1], f32, tag="cssb", name="cssb")
                nc.vector.tensor_copy(cs_sb[:], cs_t[:])
                dn_t = smp.tile([1, 4], f32, tag="sm", name="dnt")
                nc.tensor.matmul(dn_t[:], cs_sb[:], sel4_sb[:],
                                 start=True, stop=True)
                nc.vector.tensor_add(dall_sb[0:1, 4 * b:4 * b + 4],
                                     dn_t[:], exv[:])

                # PV: both pairs at once, N=256 streaming
                pv_t = pvp.tile([4, 256], f32, tag="pvt", name="pvt")
                for c in range(NCH):
                    nc.tensor.matmul(pv_t[:],
                                     pT[:, 4 * c:4 * c + 4],
                                     v_t[:, c * 256:(c + 1) * 256],
                                     start=(c == 0), stop=False)
                nc.tensor.matmul(pv_t[:], exv[:], vn_row[:],
                                 start=False, stop=True)

                pv_sb = e2.tile([4, 256], f32, tag="pvsb", name="pvsb")
                nc.vector.tensor_copy(pv_sb[:], pv_t[:])
                for pair in range(NPAIR):
                    tr = trp.tile([128, 4], f32, tag="tr", name="tr")
                    nc.tensor.transpose(tr[:],
                                        pv_sb[:, pair * 128:(pair + 1) * 128],
                                        eye4_sb[:])
                    nc.vector.tensor_copy(at_sb[pair][0:64, b:b + 1],
                                          tr[0:64, 2 * pair:2 * pair + 1])
                    nc.vector.tensor_copy(at_sb[pair][64:128, b:b + 1],
                                          tr[64:128, 2 * pair + 1:2 * pair + 2])

        # ---- phase 3: normalize + out-proj partial ----
        nc.vector.reciprocal(r_sb[:], dall_sb[:])
        with tc.tile_pool(name="rp", bufs=1, space="PSUM") as rp, \
             tc.tile_pool(name="yp", bufs=1, space="PSUM") as yp:
            for p in range(NPAIR):
                r_ps = rp.tile([128, B], f32, tag=f"rps{p}", name=f"rps{p}")
                for h in range(2):
                    j = 2 * p + h
                    nc.tensor.matmul(r_ps[:], u01_sb[0:1, h * 128:(h + 1) * 128],
                                     r_sb[0:1, j:j + 61:4],
                                     start=(h == 0), stop=(h == 1))
                nc.vector.tensor_mul(atn_sb[p][:], at_sb[p][:], r_ps[:])
            for n in range(4):
                y_ps = yp.tile([B, 512], f32, tag=f"yps{n}", name=f"yps{n}")
                for p in range(NPAIR):
                    nc.tensor.matmul(
                        y_ps[:], atn_sb[p][:],
                        wo_sb[:, p * DMODEL + n * 512: p * DMODEL + (n + 1) * 512],
                        start=(p == 0), stop=(p == NPAIR - 1))
                nc.vector.tensor_copy(y_sb[:, n * 512:(n + 1) * 512], y_ps[:])
        nc.sync.dma_start(y[:], y_sb[:])

    nc.compile()
    return nc


def _get_prog():
    global _PROG
    if _PROG is None:
        _PROG = _build()
    return _PROG


def _host_prep(x, last_pos, mask, Wq, bq, Wk, bk, Wv, bv, Wo, cache_k, cache_v):
    """Build the 8 per-core input maps."""
    f32 = np.float32
    x = np.asarray(x, f32).reshape(B, DMODEL)
    lp = np.asarray(last_pos).astype(np.int64)
    mask2 = np.asarray(mask).reshape(B, SKV).astype(bool)
    Wq = np.asarray(Wq, f32); Wk = np.asarray(Wk, f32)
    Wv = np.asarray(Wv, f32); Wo = np.asarray(Wo, f32)
    bq = np.asarray(bq, f32); bk = np.asarray(bk, f32); bv = np.asarray(bv, f32)

    xT = np.ascontiguousarray(x.T)
    _rnd = _round_f32r if STREAM_F32R else (lambda a: a)

    # shared mask tile [B, 128, 128]: dup over (pair, h); excludes stale
    # last_pos column (new-token prob handled separately, always valid)
    s_idx = np.arange(SKV)
    keep = (mask2 & (s_idx[None, :] != lp[:, None])).astype(f32)
    km = keep.reshape(B, NCH, 128).transpose(0, 2, 1)          # [B,128,32]
    m1 = np.ascontiguousarray(np.repeat(km, 4, axis=2))        # [B,128,(c,pp,h)]
    assert bool(np.all(mask2[np.arange(B), lp])), \
        "new-token position must be attendable"

    sel = np.zeros((128, 4), f32)
    m = np.arange(128)
    sel[m, m % 4] = 1.0
    u01 = np.zeros((1, 256), f32)
    u01[0, 0:64] = 1.0
    u01[0, 192:256] = 1.0
    consts = {
        "ones128": np.ones((128, 1), f32),
        "sel4": sel,
        "ones16": np.ones((1, B), f32),
        "u01": u01,
        "eye4": np.eye(4, dtype=f32),
    }

    in_maps = []
    for c in range(NCORES):
        fs = slice(c * 256, (c + 1) * 256)
        qb = bq[fs] * 0.125
        kb = bk[fs]
        ck = cache_k[:, 4 * c:4 * c + 4]
        cv = cache_v[:, 4 * c:4 * c + 4]
        in_maps.append({
            "xT": xT,
            "wqk": np.ascontiguousarray(
                np.concatenate([Wq[:, fs] * 0.125, Wk[:, fs]], axis=1)),
            "bqk": np.ascontiguousarray(
                np.stack([qb[:128], qb[128:], kb[:128], kb[128:]], axis=1)),
            "wv": np.ascontiguousarray(Wv[:, fs]),
            "bv": bv[fs].reshape(1, 256).copy(),
            "wo": np.ascontiguousarray(Wo[fs, :]),
            "kt2": _rnd(np.ascontiguousarray(
                ck.reshape(B, 2, 2, SKV, HD).transpose(0, 1, 2, 4, 3)
                .reshape(NBP, 128, SKV))),
            "vz": _rnd(np.ascontiguousarray(
                cv.reshape(B, 2, 2, SKV, HD).transpose(0, 3, 1, 2, 4)
                .reshape(B, SKV, 256))),
            "m1": m1,
            **consts,
        })
    return in_maps, lp


def _run(in_maps, trace=False):
    from concourse.bass_utils import run_bass_kernel_spmd
    nc = _get_prog()
    return run_bass_kernel_spmd(nc, in_maps, core_ids=list(range(NCORES)),
                                trace=trace)


def kernel(x, last_pos, mask, Wq, bq, Wk, bk, Wv, bv, Wo, bo,
           cache_k, cache_v, _trace=False, _result_holder=None):
    f32 = np.float32
    cache_k = np.asarray(cache_k, f32)
    cache_v = np.asarray(cache_v, f32)
    in_maps, lp = _host_prep(x, last_pos, mask, Wq, bq, Wk, bk, Wv, bv, Wo,
                             cache_k, cache_v)

    res = _run(in_maps, trace=_trace)
    if _result_holder is not None:
        _result_holder.append(res)

    bo = np.asarray(bo, f32)
    out = np.zeros((B, DMODEL), f32)
    k_new = np.zeros((B, NHEAD, HD), f32)
    v_new = np.zeros((B, NHEAD, HD), f32)
    for c in range(NCORES):
        r = res.results[c]
        out += r["y"]
        k_new[:, 4 * c:4 * c + 4] = (
            r["knew"].reshape(2, 2, HD, B).transpose(3, 0, 1, 2)
            .reshape(B, HLOC, HD))
        v_new[:, 4 * c:4 * c + 4] = r["vnew"].reshape(B, HLOC, HD)
    out = (out + bo).reshape(B, 1, DMODEL).astype(f32)

    up_k = cache_k.copy()
    up_v = cache_v.copy()
    bidx = np.arange(B)
    up_k[bidx, :, lp, :] = k_new
    up_v[bidx, :, lp, :] = v_new
    return out, up_k, up_v


# revision 23
# speedup vs baseline: 1.1242x; 1.0447x over previous
"""Bass/Tile TRN2 kernel for BlenderbotSelfAttention decode step.

B=16, QLEN=1, DMODEL=2048, NHEAD=32, HEAD_DIM=64, SKV=4096.
Sharding: tensor-parallel over heads -- 4 heads per core on 8 cores.
Heads are processed in pairs with their head_dim=64 vectors stacked to
K=128, and the big KV-cache matmuls stream the cache through the PE
moving-operand port as float32r with N>=256 (1 cycle/row), never through
LDWEIGHTS.  Scores for both pairs accumulate into one [4, 512] psum tile
(block-diagonal q with zero columns for the other pair), one exp per
512-chunk covers all 4 rows, and probabilities are transposed back to
column layout with PE-mode transposes ([4,128] -> psum [128,4]).
Row-parallel out_proj partials are summed on host (all-reduce equivalent).
"""

import numpy as np

B = 16
DMODEL = 2048
NHEAD = 32
HD = 64
SKV = 4096
NCORES = 8
HLOC = NHEAD // NCORES      # 4 heads per core
NPAIR = HLOC // 2           # 2 pairs per core
NBP = B * NPAIR             # 32 (batch, pair) units per core
NCH = SKV // 128            # 32 seq chunks of 128

_PROG = None
# stream-matmul operand mode: float32r (TF32-like, 11-bit mantissa, 1 cyc/row
# at N>=256) vs plain float32 (4 cyc/row). f32r needs host-side pre-rounding.
import os as _os
STREAM_F32R = _os.environ.get("KSTREAM", "f32r") == "f32r"


def _round_f32r(a):
    """Round-to-nearest-even to 11-bit mantissa (fp32r), fp32-bit-compatible."""
    u = np.ascontiguousarray(a, np.float32).view(np.uint32)
    r = (u + np.uint32(0x7FF) + ((u >> np.uint32(12)) & np.uint32(1))) & np.uint32(0xFFFFF000)
    return r.view(np.float32)


def _build():
    """Build + compile the per-core Bass program (identical on all cores)."""
    from contextlib import ExitStack

    import concourse.bacc as bacc
    import concourse.mybir as mybir
    import concourse.tile as tile

    f32 = mybir.dt.float32
    f32r = mybir.dt.float32r
    STR = f32r if STREAM_F32R else f32
    EXP = mybir.ActivationFunctionType.Exp
    nc = bacc.Bacc("TRN2", target_bir_lowering=False, debug=False,
                   enable_asserts=False, num_devices=NCORES)

    xT = nc.dram_tensor("xT", [DMODEL, B], f32, kind="ExternalInput").ap()
    wqk = nc.dram_tensor("wqk", [DMODEL, 512], f32, kind="ExternalInput").ap()
    bqk = nc.dram_tensor("bqk", [128, 4], f32, kind="ExternalInput").ap()
    wv = nc.dram_tensor("wv", [DMODEL, 256], f32, kind="ExternalInput").ap()
    bv = nc.dram_tensor("bv", [1, 256], f32, kind="ExternalInput").ap()
    wo = nc.dram_tensor("wo", [256, DMODEL], f32, kind="ExternalInput").ap()
    kt2 = nc.dram_tensor("kt2", [NBP, 128, SKV], STR, kind="ExternalInput").ap()
    vz = nc.dram_tensor("vz", [B, SKV, 256], STR, kind="ExternalInput").ap()
    m1 = nc.dram_tensor("m1", [B, 128, 128], STR, kind="ExternalInput").ap()
    ones128 = nc.dram_tensor("ones128", [128, 1], f32, kind="ExternalInput").ap()
    sel4 = nc.dram_tensor("sel4", [128, 4], f32, kind="ExternalInput").ap()
    ones16 = nc.dram_tensor("ones16", [1, B], f32, kind="ExternalInput").ap()
    u01 = nc.dram_tensor("u01", [1, 256], f32, kind="ExternalInput").ap()
    eye4 = nc.dram_tensor("eye4", [4, 4], f32, kind="ExternalInput").ap()

    y = nc.dram_tensor("y", [B, DMODEL], f32, kind="ExternalOutput").ap()
    knew = nc.dram_tensor("knew", [256, B], f32, kind="ExternalOutput").ap()
    vnew = nc.dram_tensor("vnew", [B, 256], f32, kind="ExternalOutput").ap()

    NK = DMODEL // 128  # 16 contraction chunks for the projections

    with tile.TileContext(nc) as tc, ExitStack() as ctx:
        cpool = ctx.enter_context(tc.tile_pool(name="cpool", bufs=1))

        # ---- small resident constants ----
        wo_sb = cpool.tile([128, 2 * DMODEL], f32, tag="wo", name="wo")
        nc.sync.dma_start(
            wo_sb[:].rearrange("p (kc f) -> p kc f", kc=2),
            wo.rearrange("(kc p) f -> p kc f", p=128))
        bqk_sb = cpool.tile([128, 4], f32, tag="bqk", name="bqk")
        nc.sync.dma_start(bqk_sb[:], bqk)
        bv_sb = cpool.tile([1, 256], f32, tag="bv", name="bv")
        nc.sync.dma_start(bv_sb[:], bv)
        m1_sb = cpool.tile([128, B * 128], STR, tag="m1", name="m1")
        nc.sync.dma_start(
            m1_sb[:].rearrange("p (b e) -> p b e", b=B),
            m1.rearrange("b p e -> p b e"))
        ones128_sb = cpool.tile([128, 1], f32, tag="ones128", name="ones128")
        nc.sync.dma_start(ones128_sb[:], ones128)
        sel4_sb = cpool.tile([128, 4], f32, tag="sel4", name="sel4")
        nc.sync.dma_start(sel4_sb[:], sel4)
        ones16_sb = cpool.tile([1, B], f32, tag="ones16", name="ones16")
        nc.sync.dma_start(ones16_sb[:], ones16)
        u01_sb = cpool.tile([1, 256], f32, tag="u01", name="u01")
        nc.sync.dma_start(u01_sb[:], u01)
        eye4_sb = cpool.tile([4, 4], f32, tag="eye4", name="eye4")
        nc.sync.dma_start(eye4_sb[:], eye4)

        # persistent intermediates
        qk_sb = [cpool.tile([128, B], f32, tag=f"qk{ft}", name=f"qk{ft}")
                 for ft in range(4)]
        v_sb = cpool.tile([B, 256], f32, tag="vsb", name="vsb")
        q4 = [cpool.tile([128, 4 * B], STR, tag=f"q4_{p}", name=f"q4_{p}")
              for p in range(NPAIR)]
        at_sb = [cpool.tile([128, B], f32, tag=f"at{p}", name=f"at{p}")
                 for p in range(NPAIR)]
        dall_sb = cpool.tile([1, 2 * NBP], f32, tag="dall", name="dall")
        r_sb = cpool.tile([1, 2 * NBP], f32, tag="rall", name="rall")
        atn_sb = [cpool.tile([128, B], f32, tag=f"atn{p}", name=f"atn{p}")
                  for p in range(NPAIR)]
        y_sb = cpool.tile([B, DMODEL], f32, tag="ysb", name="ysb")

        # ---- phase 1: projections (plain fp32 for exact k/v/q) ----
        with tc.tile_pool(name="p1", bufs=1) as p1, \
             tc.tile_pool(name="pps", bufs=2, space="PSUM") as pps, \
             tc.tile_pool(name="pvs", bufs=1, space="PSUM") as pvs:
            xt_sb = p1.tile([128, NK * B], f32, tag="xt", name="xt")
            nc.sync.dma_start(
                xt_sb[:].rearrange("p (kc f) -> p kc f", kc=NK),
                xT.rearrange("(kc p) f -> p kc f", p=128))
            wqk_sb = p1.tile([128, NK * 512], f32, tag="wqk", name="wqk")
            nc.sync.dma_start(
                wqk_sb[:].rearrange("p (kc f) -> p kc f", kc=NK),
                wqk.rearrange("(kc p) f -> p kc f", p=128))
            wv_sb = p1.tile([128, NK * 256], f32, tag="wv", name="wv")
            nc.sync.dma_start(
                wv_sb[:].rearrange("p (kc f) -> p kc f", kc=NK),
                wv.rearrange("(kc p) f -> p kc f", p=128))

            # q/k column orientation: psum[feat128, b]; ft 0,1 = q pair0/1
            # (scaled 1/8 host-side), ft 2,3 = k pair0/1
            for ft in range(4):
                ps = pps.tile([128, B], f32, tag="qkps", name="qkps")
                for kc in range(NK):
                    nc.tensor.matmul(
                        ps[:],
                        wqk_sb[:, kc * 512 + ft * 128: kc * 512 + (ft + 1) * 128],
                        xt_sb[:, kc * B:(kc + 1) * B],
                        start=(kc == 0), stop=(kc == NK - 1))
                nc.vector.tensor_scalar_add(qk_sb[ft][:], ps[:],
                                            bqk_sb[:, ft:ft + 1])
            # v row orientation: psum[b, feat256]; bias via ones-row matmul
            psv = pvs.tile([B, 256], f32, tag="vps", name="vps")
            for kc in range(NK):
                nc.tensor.matmul(psv[:], xt_sb[:, kc * B:(kc + 1) * B],
                                 wv_sb[:, kc * 256:(kc + 1) * 256],
                                 start=(kc == 0), stop=False)
            nc.tensor.matmul(psv[:], ones16_sb[:], bv_sb[:],
                             start=False, stop=True)
            nc.vector.tensor_copy(v_sb[:], psv[:])

        nc.sync.dma_start(knew[0:128, :], qk_sb[2][:])
        nc.sync.dma_start(knew[128:256, :], qk_sb[3][:])
        nc.sync.dma_start(vnew[:], v_sb[:])

        # 4-row block-diagonal q: q4[p] col 4b+(2p+h) carries q of (pair p,
        # head h) on its 64-row d block; all other columns stay zero so the
        # two pairs' score matmuls can accumulate into one [4, 512] psum.
        for p in range(NPAIR):
            nc.vector.memset(q4[p][:].bitcast(f32), 0.0)
            srcq = qk_sb[p]
            d0 = q4[p][:].rearrange("p (b four) -> p b four", four=4)
            nc.vector.tensor_copy(d0[0:64, :, 2 * p:2 * p + 1],
                                  srcq[0:64, :].rearrange("p b -> p b ()"))
            nc.vector.tensor_copy(d0[64:128, :, 2 * p + 1:2 * p + 2],
                                  srcq[64:128, :].rearrange("p b -> p b ()"))

        # ---- phase 2: attention, one batch (both head pairs) at a time ----
        with tc.tile_pool(name="kp", bufs=3) as kp, \
             tc.tile_pool(name="vp", bufs=2) as vp, \
             tc.tile_pool(name="ep", bufs=2) as ep, \
             tc.tile_pool(name="e2", bufs=2) as e2, \
             tc.tile_pool(name="vnp", bufs=2) as vnp, \
             tc.tile_pool(name="scp", bufs=2, space="PSUM") as scp, \
             tc.tile_pool(name="smp", bufs=2, space="PSUM") as smp, \
             tc.tile_pool(name="pvp", bufs=2, space="PSUM") as pvp, \
             tc.tile_pool(name="trp", bufs=2, space="PSUM") as trp:
            for b in range(B):
                v_t = vp.tile([128, NCH * 256], STR, tag="vt", name="vt")
                nc.sync.dma_start(
                    v_t[:].rearrange("p (c w) -> p c w", c=NCH),
                    vz[b].rearrange("(c p) w -> p c w", p=128))
                vn_row = vnp.tile([1, 256], f32, tag="vnr", name="vnr")
                nc.sync.dma_start(vn_row[:], v_sb[b:b + 1, :])

                # e4 rows: (pair, h); scores land here as exp(score)
                e4 = ep.tile([4, SKV], f32, tag="e4", name="e4")
                pT = e2.tile([128, 128], STR, tag="pT", name="pT")
                exv = e2.tile([1, 4], f32, tag="exv", name="exv")
                kt_t = [None, None]
                for pair in range(NPAIR):
                    kt_t[pair] = kp.tile([128, SKV], STR, tag="kt", name="kt")
                    nc.sync.dma_start(kt_t[pair][:], kt2[2 * b + pair])
                lq = [q4[p][:, 4 * b:4 * b + 4] for p in range(NPAIR)]
                for j in range(8):
                    sc = scp.tile([4, 512], f32, tag="sc", name="sc")
                    for pair in range(NPAIR):
                        nc.tensor.matmul(sc[:], lq[pair],
                                         kt_t[pair][:, j * 512:(j + 1) * 512],
                                         start=(pair == 0), stop=(pair == 1))
                    nc.scalar.activation(e4[:, j * 512:(j + 1) * 512],
                                         sc[:], EXP)
                # new-token scores for both pairs -> one [1, 4] row
                ex_ps = smp.tile([1, 4], f32, tag="sm", name="exps")
                for pair in range(NPAIR):
                    lqf = lq[pair].bitcast(f32) if STREAM_F32R else lq[pair]
                    nc.tensor.matmul(ex_ps[:], qk_sb[2 + pair][:, b:b + 1],
                                     lqf, start=(pair == 0), stop=(pair == 1))
                nc.scalar.activation(exv[:], ex_ps[:], EXP)

                # transpose probs to column layout pT[p, (c, pair, h)]
                for c in range(NCH):
                    tr = trp.tile([128, 4], f32, tag="tr", name="tr")
                    nc.tensor.transpose(tr[:], e4[:, c * 128:(c + 1) * 128],
                                        eye4_sb[:])
                    nc.vector.tensor_copy(pT[:, 4 * c:4 * c + 4], tr[:])

                # mask (kills stale last_pos column and s > last_pos)
                nc.vector.tensor_mul(pT[:], pT[:],
                                     m1_sb[:, b * 128:(b + 1) * 128])

                # denominators -> dall[0, 4b + (2*pair + h)]
                cs_t = smp.tile([128, 1], f32, tag="sm", name="cst")
                nc.tensor.matmul(cs_t[:], pT[:].bitcast(f32),
                                 ones128_sb[:], start=True, stop=True)
                cs_sb = e2.tile([128, 1], f32, tag="cssb", name="cssb")
                nc.vector.tensor_copy(cs_sb[:], cs_t[:])
                dn_t = smp.tile([1, 4], f32, tag="sm", name="dnt")
                nc.tensor.matmul(dn_t[:], cs_sb[:], sel4_sb[:],
                                 start=True, stop=True)
                nc.vector.tensor_add(dall_sb[0:1, 4 * b:4 * b + 4],
                                     dn_t[:], exv[:])

                # PV: both pairs at once, N=256 streaming
                pv_t = pvp.tile([4, 256], f32, tag="pvt", name="pvt")
                for c in range(NCH):
                    nc.tensor.matmul(pv_t[:],
                                     pT[:, 4 * c:4 * c + 4],
                                     v_t[:, c * 256:(c + 1) * 256],
                                     start=(c == 0), stop=False)
                nc.tensor.matmul(pv_t[:], exv[:], vn_row[:],
                                 start=False, stop=True)

                pv_sb = e2.tile([4, 256], f32, tag="pvsb", name="pvsb")
                nc.vector.tensor_copy(pv_sb[:], pv_t[:])
                for pair in range(NPAIR):
                    tr = trp.tile([128, 4], f32, tag="tr", name="tr")
                    nc.tensor.transpose(tr[:],
                                        pv_sb[:, pair * 128:(pair + 1) * 128],
                                        eye4_sb[:])
                    nc.vector.tensor_copy(at_sb[pair][0:64, b:b + 1],
                                          tr[0:64, 2 * pair:2 * pair + 1])
                    nc.vector.tensor_copy(at_sb[pair][64:128, b:b + 1],
                                          tr[64:128, 2 * pair + 1:2 * pair + 2])

        # ---- phase 3: normalize + out-proj partial ----
        nc.vector.reciprocal(r_sb[:], dall_sb[:])
        with tc.tile_pool(name="rp", bufs=1, space="PSUM") as rp, \
             tc.tile_pool(name="yp", bufs=1, space="PSUM") as yp:
            for p in range(NPAIR):
                r_ps = rp.tile([128, B], f32, tag=f"rps{p}", name=f"rps{p}")
                for h in range(2):
                    j = 2 * p + h
                    nc.tensor.matmul(r_ps[:], u01_sb[0:1, h * 128:(h + 1) * 128],
                                     r_sb[0:1, j:j + 61:4],
                                     start=(h == 0), stop=(h == 1))
                nc.vector.tensor_mul(atn_sb[p][:], at_sb[p][:], r_ps[:])
            for n in range(4):
                y_ps = yp.tile([B, 512], f32, tag=f"yps{n}", name=f"yps{n}")
                for p in range(NPAIR):
                    nc.tensor.matmul(
                        y_ps[:], atn_sb[p][:],
                        wo_sb[:, p * DMODEL + n * 512: p * DMODEL + (n + 1) * 512],
                        start=(p == 0), stop=(p == NPAIR - 1))
                nc.vector.tensor_copy(y_sb[:, n * 512:(n + 1) * 512], y_ps[:])
        nc.sync.dma_start(y[:], y_sb[:])

    nc.compile()
    return nc


def _get_prog():
    global _PROG
    if _PROG is None:
        _PROG = _build()
    return _PROG


def _host_prep(x, last_pos, mask, Wq, bq, Wk, bk, Wv, bv, Wo, cache_k, cache_v):
    """Build the 8 per-core input maps."""
    f32 = np.float32
    x = np.asarray(x, f32).reshape(B, DMODEL)
    lp = np.asarray(last_pos).astype(np.int64)
    mask2 = np.asarray(mask).reshape(B, SKV).astype(bool)
    Wq = np.asarray(Wq, f32); Wk = np.asarray(Wk, f32)
    Wv = np.asarray(Wv, f32); Wo = np.asarray(Wo, f32)
    bq = np.asarray(bq, f32); bk = np.asarray(bk, f32); bv = np.asarray(bv, f32)

    xT = np.ascontiguousarray(x.T)
    _rnd = _round_f32r if STREAM_F32R else (lambda a: a)

    # shared mask tile [B, 128, 128]: dup over (pair, h); excludes stale
    # last_pos column (new-token prob handled separately, always valid)
    s_idx = np.arange(SKV)
    keep = (mask2 & (s_idx[None, :] != lp[:, None])).astype(f32)
    km = keep.reshape(B, NCH, 128).transpose(0, 2, 1)          # [B,128,32]
    m1 = np.ascontiguousarray(np.repeat(km, 4, axis=2))        # [B,128,(c,pp,h)]
    assert bool(np.all(mask2[np.arange(B), lp])), \
        "new-token position must be attendable"

    sel = np.zeros((128, 4), f32)
    m = np.arange(128)
    sel[m, m % 4] = 1.0
    u01 = np.zeros((1, 256), f32)
    u01[0, 0:64] = 1.0
    u01[0, 192:256] = 1.0
    consts = {
        "ones128": np.ones((128, 1), f32),
        "sel4": sel,
        "ones16": np.ones((1, B), f32),
        "u01": u01,
        "eye4": np.eye(4, dtype=f32),
    }

    in_maps = []
    for c in range(NCORES):
        fs = slice(c * 256, (c + 1) * 256)
        qb = bq[fs] * 0.125
        kb = bk[fs]
        ck = cache_k[:, 4 * c:4 * c + 4]
        cv = cache_v[:, 4 * c:4 * c + 4]
        in_maps.append({
            "xT": xT,
            "wqk": np.ascontiguousarray(
                np.concatenate([Wq[:, fs] * 0.125, Wk[:, fs]], axis=1)),
            "bqk": np.ascontiguousarray(
                np.stack([qb[:128], qb[128:], kb[:128], kb[128:]], axis=1)),
            "wv": np.ascontiguousarray(Wv[:, fs]),
            "bv": bv[fs].reshape(1, 256).copy(),
            "wo": np.ascontiguousarray(Wo[fs, :]),
            "kt2": _rnd(np.ascontiguousarray(
                ck.reshape(B, 2, 2, SKV, HD).transpose(0, 1, 2, 4, 3)
                .reshape(NBP, 128, SKV))),
            "vz": _rnd(np.ascontiguousarray(
                cv.reshape(B, 2, 2, SKV, HD).transpose(0, 3, 1, 2, 4)
                .reshape(B, SKV, 256))),
            "m1": m1,
            **consts,
        })
    return in_maps, lp


def _run(in_maps, trace=False):
    from concourse.bass_utils import run_bass_kernel_spmd
    nc = _get_prog()
    return run_bass_kernel_spmd(nc, in_maps, core_ids=list(range(NCORES)),
                                trace=trace)


def kernel(x, last_pos, mask, Wq, bq, Wk, bk, Wv, bv, Wo, bo,
           cache_k, cache_v, _trace=False, _result_holder=None):
    f32 = np.float32
    cache_k = np.asarray(cache_k, f32)
    cache_v = np.asarray(cache_v, f32)
    in_maps, lp = _host_prep(x, last_pos, mask, Wq, bq, Wk, bk, Wv, bv, Wo,
                             cache_k, cache_v)

    res = _run(in_maps, trace=_trace)
    if _result_holder is not None:
        _result_holder.append(res)

    bo = np.asarray(bo, f32)
    out = np.zeros((B, DMODEL), f32)
    k_new = np.zeros((B, NHEAD, HD), f32)
    v_new = np.zeros((B, NHEAD, HD), f32)
    for c in range(NCORES):
        r = res.results[c]
        out += r["y"]
        k_new[:, 4 * c:4 * c + 4] = (
            r["knew"].reshape(2, 2, HD, B).transpose(3, 0, 1, 2)
            .reshape(B, HLOC, HD))
        v_new[:, 4 * c:4 * c + 4] = r["vnew"].reshape(B, HLOC, HD)
    out = (out + bo).reshape(B, 1, DMODEL).astype(f32)

    up_k = cache_k.copy()
    up_v = cache_v.copy()
    bidx = np.arange(B)
    up_k[bidx, :, lp, :] = k_new
    up_v[bidx, :, lp, :] = v_new
    return out, up_k, up_v
